# revision 33
# baseline (speedup 1.0000x reference)
"""GQA attention (B=1, E=4096, H=32, KVH=8, S=2048, HD=128) on 8 TRN2 cores.

Sharding: tensor-parallel over heads. Core c owns q heads {c, c+8, c+16, c+24}
(all of which attend to kv head c under the reference's channel-block-repeat
GQA tiling), so each core computes its 4 q-head projections + 1 kv-head k/v
projection + RoPE + causal attention entirely locally. Attention outputs
(4 heads x [128, S] each) are AllGathered (2 MiB/rank bf16), after which each
core computes a 512-row slice of the o_proj output. Host concatenates slices.

Numerics: matmuls in bf16 (f32 PSUM accumulation); softmax without
max-subtraction (scores are O(5), exp is safe in f32); exp on ScalarE in f32
from PSUM with the 1/sqrt(128) scale folded into the activation scale.
"""

import numpy as np
import ml_dtypes

B, E, H, KVH, S = 1, 4096, 32, 8, 2048
HD = E // H            # 128
NCORES = 8
QH = H // NCORES       # 4 q heads per core
EB = E // 128          # 32 e-blocks
ST = S // 512          # 4 s-tiles of 512
JB = S // 128          # 16 j-blocks of 128
OCB = (E // NCORES) // 128  # 4 output-channel blocks per core (512 rows)

_BF16 = ml_dtypes.bfloat16

_COMPILED = None       # (nc, names) cache
LAST_EXEC_NS = None    # set when _profile=True
LAST_INSTS = None


def _build_graph():
    import concourse.bass as bass
    import concourse.bacc as bacc
    import concourse.mybir as mybir
    from concourse import tile
    from concourse.masks import make_identity

    f32 = mybir.dt.float32
    bf16 = mybir.dt.bfloat16
    SCALE = 1.0 / float(np.sqrt(HD))

    nc = bacc.Bacc("TRN2", target_bir_lowering=False, num_devices=NCORES)

    xp = nc.declare_dram_parameter("xp", [ST, 128, EB, 512], bf16, isOutput=False)
    wqt = nc.declare_dram_parameter("wqt", [128, EB, QH, 128], bf16, isOutput=False)
    wkt = nc.declare_dram_parameter("wkt", [128, EB, 128], bf16, isOutput=False)
    wvt = nc.declare_dram_parameter("wvt", [128, EB, 128], bf16, isOutput=False)
    wot = nc.declare_dram_parameter("wot", [128, EB, OCB, 128], bf16, isOutput=False)
    cosd = nc.declare_dram_parameter("cosd", [128, S], f32, isOutput=False)
    sind = nc.declare_dram_parameter("sind", [128, S], f32, isOutput=False)
    bvp = nc.declare_dram_parameter("bvp", [128, 1], f32, isOutput=False)
    bop = nc.declare_dram_parameter("bop", [128, OCB], f32, isOutput=False)
    out = nc.declare_dram_parameter("out", [OCB, ST, 128, 512], f32, isOutput=True)

    with tile.TileContext(nc) as tc:
        with (
            tc.tile_pool(name="const", bufs=1) as constp,
            tc.tile_pool(name="keep", bufs=1) as keep,
            tc.tile_pool(name="dramp", bufs=1, space="DRAM") as dramp,
        ):
            # --- constants on-chip ---
            ident = constp.tile([128, 128], bf16)
            make_identity(nc, ident)
            ones = constp.tile([128, 128], bf16)
            nc.gpsimd.memset(ones[:], 1.0)
            # diag masks: mask_d[j, s] = 1 if s >= 128*d + j else 0
            dmask = constp.tile([128, 4, 512], bf16)
            for d in range(4):
                nc.gpsimd.memset(dmask[:, d, :], 1.0)
                nc.gpsimd.affine_select(
                    out=dmask[:, d, :], in_=dmask[:, d, :],
                    compare_op=mybir.AluOpType.is_ge,
                    fill=0.0, base=-128 * d,
                    pattern=[[1, 512]], channel_multiplier=-1,
                )
            cos_s = constp.tile([128, S], f32)
            sin_s = constp.tile([128, S], f32)
            nc.gpsimd.dma_start(cos_s[:], cosd[:])
            nc.gpsimd.dma_start(sin_s[:], sind[:])
            bv_s = constp.tile([128, 1], f32)
            nc.gpsimd.dma_start(bv_s[:], bvp[:])
            bo_s = constp.tile([128, OCB], f32)
            nc.gpsimd.dma_start(bo_s[:], bop[:])

            # --- outputs of phase A kept for attention ---
            q_sb = keep.tile([128, QH, S], bf16)     # roped q per head (p=hd channel)
            k_sb = keep.tile([128, S], bf16)         # roped k
            vT_sb = keep.tile([128, JB, 128], bf16)  # v transposed blocks (p=key j)
            o_sb = keep.tile([128, QH, S], bf16)     # attention outputs

            # collective buffers (one AllGather per q-head, overlapped with attention)
            cc_in_h = [dramp.tile([128, S], bf16, name=f"ccin{hi}") for hi in range(QH)]
            cc_out_h = [dramp.tile([NCORES, 128, S], bf16, addr_space="Shared", name=f"ccout{hi}")
                        for hi in range(QH)]

            # ---------------- Phase A: QKV projection + rope ----------------
            with (
                tc.tile_pool(name="aw", bufs=1) as aw,
                tc.tile_pool(name="ax", bufs=2) as ax,
                tc.tile_pool(name="atmp", bufs=2) as atmp,
                tc.tile_pool(name="apsum", bufs=1, space="PSUM") as apsum,
            ):
                xch0 = []
                wk_s = aw.tile([128, EB, 128], bf16)
                wv_s = aw.tile([128, EB, 128], bf16)
                wq_s = aw.tile([128, EB, QH, 128], bf16)
                for cix in range(4):
                    cs = slice(cix * 8, (cix + 1) * 8)
                    xc = ax.tile([128, 8, 512], bf16, tag="x", bufs=6, name=f"xc0_{cix}")
                    nc.sync.dma_start(xc[:], xp[0, :, cs, :])
                    xch0.append(xc)
                    nc.sync.dma_start(wk_s[:, cs, :], wkt[:, cs, :])
                    nc.sync.dma_start(wv_s[:, cs, :], wvt[:, cs, :])
                    nc.sync.dma_start(wq_s[:, cs, :, :], wqt[:, cs, :, :])

                for t in range(ST):
                    sl = slice(t * 512, (t + 1) * 512)
                    if t == 0:
                        xch = xch0
                    else:
                        xch = []
                        for cix in range(4):
                            xc = ax.tile([128, 8, 512], bf16, tag="x", bufs=6)
                            nc.sync.dma_start(xc[:], xp[t, :, cix * 8:(cix + 1) * 8, :])
                            xch.append(xc)

                    ps = []
                    for hi in range(QH):
                        p = apsum.tile([128, 512], f32, tag=f"q{hi}")
                        ps.append(p)
                    k_ps = apsum.tile([128, 512], f32, tag="k")
                    v_ps = apsum.tile([128, 512], f32, tag="v")

                    for b in range(EB):
                        st, sp = (b == 0), (b == EB - 1)
                        xb = xch[b // 8][:, b % 8, :]
                        for hi in range(QH):
                            nc.tensor.matmul(ps[hi][:], wq_s[:, b, hi, :], xb, start=st, stop=sp)
                        nc.tensor.matmul(k_ps[:], wk_s[:, b, :], xb, start=st, stop=sp)
                        nc.tensor.matmul(v_ps[:], wv_s[:, b, :], xb, start=st, stop=sp)

                    # rope for q heads and k: out[0:64]  = p[0:64]*cos[0:64]   - p[64:]*sin[0:64]
                    #                         out[64:]   = p[64:]*cos[64:]     + p[0:64]*sin[64:]
                    # rope: dst = p*cos + swap_halves(p)*sinmod, where
                    # sinmod[0:64] = -sin[0:64] (host-side sign fold).
                    for src, dst in [(ps[0], q_sb[:, 0, :]), (ps[1], q_sb[:, 1, :]),
                                     (ps[2], q_sb[:, 2, :]), (ps[3], q_sb[:, 3, :]),
                                     (k_ps, k_sb[:])]:
                        qf = atmp.tile([128, 512], f32, tag="qf")
                        nc.scalar.copy(qf[:], src[:])
                        rot = atmp.tile([128, 512], f32, tag="rot")
                        nc.sync.dma_start(rot[0:64, :], qf[64:128, :])
                        nc.sync.dma_start(rot[64:128, :], qf[0:64, :])
                        qc = atmp.tile([128, 512], f32, tag="qc")
                        nc.vector.tensor_mul(qc[:], src[:], cos_s[:, sl])
                        rs = atmp.tile([128, 512], f32, tag="rs")
                        nc.vector.tensor_mul(rs[:], rot[:], sin_s[:, sl])
                        nc.vector.tensor_add(dst[:, sl], qc[:], rs[:])

                    # v: add bias, cast to bf16, then transpose 128-blocks
                    v_sb = atmp.tile([128, 512], bf16, tag="v")
                    nc.vector.tensor_scalar_add(v_sb[:], v_ps[:], bv_s[:, 0:1])
                    for i in range(4):
                        jb = 4 * t + i
                        tr_ps = apsum.tile([128, 128], bf16, tag="tr")
                        nc.tensor.transpose(tr_ps[:], v_sb[:, i * 128:(i + 1) * 128], ident[:])
                        nc.vector.tensor_copy(vT_sb[:, jb, :], tr_ps[:])

            # ------- Phase B+C: attention, per-head AllGather, o_proj partials -------
            # Emission staggered so o_proj(hi) matmuls (ready once AG(hi) lands)
            # fill PE idle during the ACT-paced exp of attention(hi+1).
            with (
                tc.tile_pool(name="bw", bufs=3) as bw,
                tc.tile_pool(name="btmp", bufs=2) as btmp,
                tc.tile_pool(name="bpsum", bufs=1, space="PSUM") as bpsum,
                tc.tile_pool(name="cw", bufs=1) as cw,
                tc.tile_pool(name="cg", bufs=2) as cg,
                tc.tile_pool(name="cout", bufs=1) as cout,
                tc.tile_pool(name="cpsum", space="PSUM", bufs=2) as cpsum,
            ):
                wo_s = cw.tile([128, EB, OCB, 128], bf16)
                out_acc = cout.tile([128, ST, OCB, 512], f32)

                def emit_attention(hi):
                    for t in range(ST):
                        sl = slice(t * 512, (t + 1) * 512)
                        njb = 4 * t + 4
                        attn_ps = bpsum.tile([128, 512], f32, tag="attn", name=f"at{hi}{t}")
                        sums_ps = bpsum.tile([128, 512], f32, tag="sums", name=f"su{hi}{t}")
                        for pj in range(njb // 2):
                            jb0 = 2 * pj
                            sc_ps = bpsum.tile([128, 1024], f32, tag="sc", bufs=2, name=f"sc{hi}{t}{pj}")
                            # diag blocks: only columns >= 128*d are unmasked; shrink N.
                            # (hi==0,t==0 keeps full N so both psum slots are written
                            # before their first exp — avoids NaN from uninit PSUM.)
                            def c0(jb):
                                d = jb - 4 * t
                                return 0 if (d < 0 or (hi == 0 and t == 0)) else 128 * d
                            for u in range(2):
                                jb = jb0 + u
                                nc.tensor.matmul(sc_ps[:, u * 512 + c0(jb):(u + 1) * 512],
                                                 k_sb[:, jb * 128:(jb + 1) * 128],
                                                 q_sb[:, hi, t * 512 + c0(jb):(t + 1) * 512],
                                                 start=True, stop=True)
                            w_t = bw.tile([128, 1024], bf16, tag="wt", name=f"wt{hi}{t}{pj}")
                            cmin = c0(jb0)
                            nc.scalar.activation(w_t[:, cmin:1024], sc_ps[:, cmin:1024],
                                                 mybir.ActivationFunctionType.Exp, scale=SCALE)
                            for u in range(2):
                                jb = jb0 + u
                                d = jb - 4 * t
                                cc = u * 512 + c0(jb)
                                if d >= 0:
                                    nc.vector.tensor_mul(w_t[:, cc:(u + 1) * 512], w_t[:, cc:(u + 1) * 512],
                                                         dmask[:, d, c0(jb):512])
                                st, sp = (jb == 0), (jb == njb - 1)
                                nc.tensor.matmul(attn_ps[:, c0(jb):512], vT_sb[:, jb, :], w_t[:, cc:(u + 1) * 512],
                                                 start=st, stop=sp, skip_group_check=True)
                                nc.tensor.matmul(sums_ps[:, c0(jb):512], ones[:], w_t[:, cc:(u + 1) * 512],
                                                 start=st, stop=sp, skip_group_check=True)
                        recip = btmp.tile([128, 512], f32, tag="recip", name=f"re{hi}{t}")
                        nc.vector.reciprocal_approx_fast(recip[:], sums_ps[:])
                        nc.vector.tensor_mul(o_sb[:, hi, sl], attn_ps[:], recip[:])
                        nc.gpsimd.dma_start(cc_in_h[hi][:, sl], o_sb[:, hi, sl])
                    nc.gpsimd.collective_compute(
                        "AllGather",
                        mybir.AluOpType.bypass,
                        replica_groups=[list(range(NCORES))],
                        ins=[cc_in_h[hi][:]],
                        outs=[cc_out_h[hi][:]],
                    )

                def emit_oproj_partial(hi):
                    for t in range(ST):
                        ogc = cg.tile([128, NCORES, 512], bf16, tag="og", bufs=6, name=f"og{hi}{t}")
                        nc.sync.dma_start(
                            ogc[:], cc_out_h[hi][:, :, t * 512:(t + 1) * 512].transpose([1, 0, 2]))
                        for o in range(OCB):
                            o_ps = cpsum.tile([128, 512], f32, tag="ops", name=f"op{hi}{t}{o}")
                            for r in range(NCORES):
                                nc.tensor.matmul(o_ps[:], wo_s[:, hi * NCORES + r, o, :], ogc[:, r, :],
                                                 start=(r == 0), stop=(r == NCORES - 1))
                            if hi == 0:
                                nc.vector.tensor_copy(out_acc[:, t, o, :], o_ps[:])
                            else:
                                nc.vector.tensor_add(out_acc[:, t, o, :], out_acc[:, t, o, :], o_ps[:])
                            if hi == QH - 1:
                                nc.vector.tensor_scalar_add(out_acc[:, t, o, :], out_acc[:, t, o, :], bo_s[:, o:o + 1])
                                nc.scalar.dma_start(out[o, t], out_acc[:, t, o, :])

                emit_attention(0)
                for cix in range(4):
                    nc.sync.dma_start(wo_s[:, cix * 8:(cix + 1) * 8, :, :],
                                      wot[:, cix * 8:(cix + 1) * 8, :, :])
                emit_attention(1)
                emit_attention(2)
                emit_attention(3)
                emit_oproj_partial(0)
                emit_oproj_partial(1)
                emit_oproj_partial(2)
                emit_oproj_partial(3)

    nc.finalize()
    return nc


def _pack_inputs(inputs):
    """Host-side shard + pack into DMA-friendly per-core layouts."""
    x = np.asarray(inputs["input_embeds"], np.float32).reshape(E, S)
    cos = np.asarray(inputs["cos"], np.float32)
    sin = np.asarray(inputs["sin"], np.float32)
    wq = np.asarray(inputs["wq"], np.float32)
    wk = np.asarray(inputs["wk"], np.float32)
    wv = np.asarray(inputs["wv"], np.float32)
    bv = np.asarray(inputs["bv"], np.float32)
    wo = np.asarray(inputs["wo"], np.float32)
    bo = np.asarray(inputs["bo"], np.float32)

    sinmod = np.concatenate([-sin[:64], sin[64:]], axis=0)

    # x packed: [ST, 128, EB, 512]; xp[t, p, b, s] = x[b*128+p, t*512+s]
    xp = np.ascontiguousarray(
        x.reshape(EB, 128, ST, 512).transpose(2, 1, 0, 3)
    ).astype(_BF16)

    # gathered-channel permutation for wo columns (hi-major after per-head AG):
    # g = hi*1024 + r*128 + d -> original channel (r + 8*hi)*128 + d
    g = np.arange(E)
    hi, rem = g // (NCORES * 128), g % (NCORES * 128)
    r, d = rem // 128, rem % 128
    colperm = (r + NCORES * hi) * 128 + d

    in_maps = []
    for c in range(NCORES):
        qheads = [c + NCORES * i for i in range(QH)]
        # wqt[p, b, hi, m] = wq[head*128+m, b*128+p]
        wq_loc = wq[np.concatenate([np.arange(h * 128, (h + 1) * 128) for h in qheads])]  # [512, E]
        wqt = np.ascontiguousarray(
            wq_loc.reshape(QH, 128, EB, 128).transpose(3, 2, 0, 1)
        ).astype(_BF16)
        wk_loc = wk[c * 128:(c + 1) * 128]  # [128, E]
        wkt = np.ascontiguousarray(
            wk_loc.reshape(128, EB, 128).transpose(2, 1, 0)
        ).astype(_BF16)
        wv_loc = wv[c * 128:(c + 1) * 128]
        wvt = np.ascontiguousarray(
            wv_loc.reshape(128, EB, 128).transpose(2, 1, 0)
        ).astype(_BF16)
        # wot[p, b, o, m] = wo[c*512 + o*128 + m, colperm[b*128+p]]
        wo_loc = wo[c * 512:(c + 1) * 512][:, colperm]  # [512, E] permuted cols
        wot = np.ascontiguousarray(
            wo_loc.reshape(OCB, 128, EB, 128).transpose(3, 2, 0, 1)
        ).astype(_BF16)
        in_maps.append({
            "xp": xp,
            "wqt": wqt, "wkt": wkt, "wvt": wvt, "wot": wot,
            "cosd": cos, "sind": sinmod,
            "bvp": np.ascontiguousarray(bv[c * 128:(c + 1) * 128].reshape(128, 1)),
            "bop": np.ascontiguousarray(bo[c * 512:(c + 1) * 512].reshape(OCB, 128).T),
        })
    return in_maps


def _install_ntff_hook():
    """The agent image lacks antenv.axon_hooks; recreate it so trace=True
    (neuron-profile exec_time_ns) works under axon."""
    import sys, types
    try:
        from antenv.axon_hooks import get_axon_ntff_profile_hook  # noqa
        return
    except ImportError:
        pass
    mod = types.ModuleType("antenv.axon_hooks")
    _h = [None]
    mod.set_axon_ntff_profile_hook = lambda h: _h.__setitem__(0, h)
    mod.get_axon_ntff_profile_hook = lambda: _h[0]
    sys.modules["antenv.axon_hooks"] = mod
    import antenv
    antenv.axon_hooks = mod
    try:
        from trn_agent_boot.trn_boot import _ntff_profile_via_ctypes
        mod.set_axon_ntff_profile_hook(
            _ntff_profile_via_ctypes("/opt/axon/libaxon_pjrt.so"))
    except Exception:
        pass


def kernel(_profile=False, **inputs):
    global _COMPILED, LAST_EXEC_NS
    from concourse.bass_utils import run_bass_kernel_spmd

    if _profile:
        _install_ntff_hook()

    if _COMPILED is None:
        _COMPILED = _build_graph()
    nc = _COMPILED

    in_maps = _pack_inputs(inputs)
    res = run_bass_kernel_spmd(nc, in_maps, core_ids=list(range(NCORES)), trace=_profile)
    if _profile:
        LAST_EXEC_NS = res.exec_time_ns
        global LAST_INSTS
        LAST_INSTS = res.instructions_and_trace
    outs = res.results

    full = np.empty((E, S), np.float32)
    for c in range(NCORES):
        oc = np.asarray(outs[c]["out"], np.float32)  # [OCB, ST, 128, 512]
        full[c * 512:(c + 1) * 512] = oc.transpose(0, 2, 1, 3).reshape(512, S)
    return full.reshape(B, E, 1, S)


# revision 34
# speedup vs baseline: 1.0026x; 1.0026x over previous
"""GQA attention (B=1, E=4096, H=32, KVH=8, S=2048, HD=128) on 8 TRN2 cores.

Sharding: tensor-parallel over heads. Core c owns q heads {c, c+8, c+16, c+24}
(all of which attend to kv head c under the reference's channel-block-repeat
GQA tiling), so each core computes its 4 q-head projections + 1 kv-head k/v
projection + RoPE + causal attention entirely locally. Attention outputs
(4 heads x [128, S] each) are AllGathered (2 MiB/rank bf16), after which each
core computes a 512-row slice of the o_proj output. Host concatenates slices.

Numerics: matmuls in bf16 (f32 PSUM accumulation); softmax without
max-subtraction (scores are O(5), exp is safe in f32); exp on ScalarE in f32
from PSUM with the 1/sqrt(128) scale folded into the activation scale.
"""

import numpy as np
import ml_dtypes

B, E, H, KVH, S = 1, 4096, 32, 8, 2048
HD = E // H            # 128
NCORES = 8
QH = H // NCORES       # 4 q heads per core
EB = E // 128          # 32 e-blocks
ST = S // 512          # 4 s-tiles of 512
JB = S // 128          # 16 j-blocks of 128
OCB = (E // NCORES) // 128  # 4 output-channel blocks per core (512 rows)

_BF16 = ml_dtypes.bfloat16

_COMPILED = None       # (nc, names) cache
LAST_EXEC_NS = None    # set when _profile=True
LAST_INSTS = None


def _build_graph():
    import concourse.bass as bass
    import concourse.bacc as bacc
    import concourse.mybir as mybir
    from concourse import tile
    from concourse.masks import make_identity

    f32 = mybir.dt.float32
    bf16 = mybir.dt.bfloat16
    SCALE = 1.0 / float(np.sqrt(HD))

    nc = bacc.Bacc("TRN2", target_bir_lowering=False, num_devices=NCORES)

    xp = nc.declare_dram_parameter("xp", [ST, 128, EB, 512], bf16, isOutput=False)
    wqt = nc.declare_dram_parameter("wqt", [128, EB, QH, 128], bf16, isOutput=False)
    wkt = nc.declare_dram_parameter("wkt", [128, EB, 128], bf16, isOutput=False)
    wvt = nc.declare_dram_parameter("wvt", [128, EB, 128], bf16, isOutput=False)
    wot = nc.declare_dram_parameter("wot", [128, EB, OCB, 128], bf16, isOutput=False)
    cosd = nc.declare_dram_parameter("cosd", [128, S], f32, isOutput=False)
    sind = nc.declare_dram_parameter("sind", [128, S], f32, isOutput=False)
    bvp = nc.declare_dram_parameter("bvp", [128, 1], f32, isOutput=False)
    bop = nc.declare_dram_parameter("bop", [128, OCB], f32, isOutput=False)
    out = nc.declare_dram_parameter("out", [OCB, ST, 128, 512], f32, isOutput=True)

    with tile.TileContext(nc) as tc:
        with (
            tc.tile_pool(name="const", bufs=1) as constp,
            tc.tile_pool(name="keep", bufs=1) as keep,
            tc.tile_pool(name="dramp", bufs=1, space="DRAM") as dramp,
        ):
            # --- constants on-chip ---
            ident = constp.tile([128, 128], bf16)
            make_identity(nc, ident)
            ones = constp.tile([128, 128], bf16)
            nc.gpsimd.memset(ones[:], 1.0)
            # diag masks: mask_d[j, s] = 1 if s >= 128*d + j else 0
            dmask = constp.tile([128, 4, 512], bf16)
            for d in range(4):
                nc.gpsimd.memset(dmask[:, d, :], 1.0)
                nc.gpsimd.affine_select(
                    out=dmask[:, d, :], in_=dmask[:, d, :],
                    compare_op=mybir.AluOpType.is_ge,
                    fill=0.0, base=-128 * d,
                    pattern=[[1, 512]], channel_multiplier=-1,
                )
            cos_s = constp.tile([128, S], f32)
            sin_s = constp.tile([128, S], f32)
            nc.gpsimd.dma_start(cos_s[:], cosd[:])
            nc.gpsimd.dma_start(sin_s[:], sind[:])
            bv_s = constp.tile([128, 1], f32)
            nc.gpsimd.dma_start(bv_s[:], bvp[:])
            bo_s = constp.tile([128, OCB], f32)
            nc.gpsimd.dma_start(bo_s[:], bop[:])

            # --- outputs of phase A kept for attention ---
            q_sb = keep.tile([128, QH, S], bf16)     # roped q per head (p=hd channel)
            k_sb = keep.tile([128, S], bf16)         # roped k
            vT_sb = keep.tile([128, JB, 128], bf16)  # v transposed blocks (p=key j)
            o_sb = keep.tile([128, QH, S], bf16)     # attention outputs

            # collective buffers (one AllGather per q-head, overlapped with attention)
            cc_in_h = [dramp.tile([128, S], bf16, name=f"ccin{hi}") for hi in range(QH)]
            cc_out_h = [dramp.tile([NCORES, 128, S], bf16, addr_space="Shared", name=f"ccout{hi}")
                        for hi in range(QH)]

            # ---------------- Phase A: QKV projection + rope ----------------
            with (
                tc.tile_pool(name="aw", bufs=1) as aw,
                tc.tile_pool(name="ax", bufs=2) as ax,
                tc.tile_pool(name="atmp", bufs=2) as atmp,
                tc.tile_pool(name="apsum", bufs=1, space="PSUM") as apsum,
            ):
                xch0 = []
                wk_s = aw.tile([128, EB, 128], bf16)
                wv_s = aw.tile([128, EB, 128], bf16)
                wq_s = aw.tile([128, EB, QH, 128], bf16)
                for cix in range(4):
                    cs = slice(cix * 8, (cix + 1) * 8)
                    xc = ax.tile([128, 8, 512], bf16, tag="x", bufs=6, name=f"xc0_{cix}")
                    nc.sync.dma_start(xc[:], xp[0, :, cs, :])
                    xch0.append(xc)
                    nc.sync.dma_start(wk_s[:, cs, :], wkt[:, cs, :])
                    nc.sync.dma_start(wv_s[:, cs, :], wvt[:, cs, :])
                    nc.sync.dma_start(wq_s[:, cs, :, :], wqt[:, cs, :, :])

                for t in range(ST):
                    sl = slice(t * 512, (t + 1) * 512)
                    if t == 0:
                        xch = xch0
                    else:
                        xch = []
                        for cix in range(4):
                            xc = ax.tile([128, 8, 512], bf16, tag="x", bufs=6)
                            nc.sync.dma_start(xc[:], xp[t, :, cix * 8:(cix + 1) * 8, :])
                            xch.append(xc)

                    ps = []
                    for hi in range(QH):
                        p = apsum.tile([128, 512], f32, tag=f"q{hi}")
                        ps.append(p)
                    k_ps = apsum.tile([128, 512], f32, tag="k")
                    v_ps = apsum.tile([128, 512], f32, tag="v")

                    for b in range(EB):
                        st, sp = (b == 0), (b == EB - 1)
                        xb = xch[b // 8][:, b % 8, :]
                        for hi in range(QH):
                            nc.tensor.matmul(ps[hi][:], wq_s[:, b, hi, :], xb, start=st, stop=sp)
                        nc.tensor.matmul(k_ps[:], wk_s[:, b, :], xb, start=st, stop=sp)
                        nc.tensor.matmul(v_ps[:], wv_s[:, b, :], xb, start=st, stop=sp)

                    # rope for q heads and k: out[0:64]  = p[0:64]*cos[0:64]   - p[64:]*sin[0:64]
                    #                         out[64:]   = p[64:]*cos[64:]     + p[0:64]*sin[64:]
                    # rope: dst = p*cos + swap_halves(p)*sinmod, where
                    # sinmod[0:64] = -sin[0:64] (host-side sign fold).
                    for src, dst in [(ps[0], q_sb[:, 0, :]), (ps[1], q_sb[:, 1, :]),
                                     (ps[2], q_sb[:, 2, :]), (ps[3], q_sb[:, 3, :]),
                                     (k_ps, k_sb[:])]:
                        qf = atmp.tile([128, 512], f32, tag="qf")
                        nc.vector.tensor_copy(qf[:], src[:])
                        rot = atmp.tile([128, 512], f32, tag="rot")
                        nc.sync.dma_start(rot[0:64, :], qf[64:128, :])
                        nc.sync.dma_start(rot[64:128, :], qf[0:64, :])
                        qc = atmp.tile([128, 512], f32, tag="qc")
                        nc.vector.tensor_mul(qc[:], src[:], cos_s[:, sl])
                        rs = atmp.tile([128, 512], f32, tag="rs")
                        nc.vector.tensor_mul(rs[:], rot[:], sin_s[:, sl])
                        nc.vector.tensor_add(dst[:, sl], qc[:], rs[:])

                    # v: add bias, cast to bf16, then transpose 128-blocks
                    v_sb = atmp.tile([128, 512], bf16, tag="v")
                    nc.vector.tensor_scalar_add(v_sb[:], v_ps[:], bv_s[:, 0:1])
                    for i in range(4):
                        jb = 4 * t + i
                        tr_ps = apsum.tile([128, 128], bf16, tag="tr")
                        nc.tensor.transpose(tr_ps[:], v_sb[:, i * 128:(i + 1) * 128], ident[:])
                        nc.vector.tensor_copy(vT_sb[:, jb, :], tr_ps[:])

            # ------- Phase B+C: attention, per-head AllGather, o_proj partials -------
            # Emission staggered so o_proj(hi) matmuls (ready once AG(hi) lands)
            # fill PE idle during the ACT-paced exp of attention(hi+1).
            with (
                tc.tile_pool(name="bw", bufs=3) as bw,
                tc.tile_pool(name="btmp", bufs=2) as btmp,
                tc.tile_pool(name="bpsum", bufs=1, space="PSUM") as bpsum,
                tc.tile_pool(name="cw", bufs=1) as cw,
                tc.tile_pool(name="cg", bufs=2) as cg,
                tc.tile_pool(name="cout", bufs=1) as cout,
                tc.tile_pool(name="cpsum", space="PSUM", bufs=2) as cpsum,
            ):
                wo_s = cw.tile([128, EB, OCB, 128], bf16)
                out_acc = cout.tile([128, ST, OCB, 512], f32)

                def emit_attention(hi):
                    for t in range(ST):
                        sl = slice(t * 512, (t + 1) * 512)
                        njb = 4 * t + 4
                        attn_ps = bpsum.tile([128, 512], f32, tag="attn", name=f"at{hi}{t}")
                        sums_ps = bpsum.tile([128, 512], f32, tag="sums", name=f"su{hi}{t}")
                        for pj in range(njb // 2):
                            jb0 = 2 * pj
                            sc_ps = bpsum.tile([128, 1024], f32, tag="sc", bufs=2, name=f"sc{hi}{t}{pj}")
                            # diag blocks: only columns >= 128*d are unmasked; shrink N.
                            # (hi==0,t==0 keeps full N so both psum slots are written
                            # before their first exp — avoids NaN from uninit PSUM.)
                            def c0(jb):
                                d = jb - 4 * t
                                return 0 if (d < 0 or (hi == 0 and t == 0)) else 128 * d
                            for u in range(2):
                                jb = jb0 + u
                                nc.tensor.matmul(sc_ps[:, u * 512 + c0(jb):(u + 1) * 512],
                                                 k_sb[:, jb * 128:(jb + 1) * 128],
                                                 q_sb[:, hi, t * 512 + c0(jb):(t + 1) * 512],
                                                 start=True, stop=True)
                            w_t = bw.tile([128, 1024], bf16, tag="wt", name=f"wt{hi}{t}{pj}")
                            cmin = c0(jb0)
                            nc.scalar.activation(w_t[:, cmin:1024], sc_ps[:, cmin:1024],
                                                 mybir.ActivationFunctionType.Exp, scale=SCALE)
                            for u in range(2):
                                jb = jb0 + u
                                d = jb - 4 * t
                                cc = u * 512 + c0(jb)
                                if d >= 0:
                                    nc.vector.tensor_mul(w_t[:, cc:(u + 1) * 512], w_t[:, cc:(u + 1) * 512],
                                                         dmask[:, d, c0(jb):512])
                                st, sp = (jb == 0), (jb == njb - 1)
                                nc.tensor.matmul(attn_ps[:, c0(jb):512], vT_sb[:, jb, :], w_t[:, cc:(u + 1) * 512],
                                                 start=st, stop=sp, skip_group_check=True)
                                nc.tensor.matmul(sums_ps[:, c0(jb):512], ones[:], w_t[:, cc:(u + 1) * 512],
                                                 start=st, stop=sp, skip_group_check=True)
                        recip = btmp.tile([128, 512], f32, tag="recip", name=f"re{hi}{t}")
                        nc.vector.reciprocal_approx_fast(recip[:], sums_ps[:])
                        nc.vector.tensor_mul(o_sb[:, hi, sl], attn_ps[:], recip[:])
                        nc.gpsimd.dma_start(cc_in_h[hi][:, sl], o_sb[:, hi, sl])
                    nc.gpsimd.collective_compute(
                        "AllGather",
                        mybir.AluOpType.bypass,
                        replica_groups=[list(range(NCORES))],
                        ins=[cc_in_h[hi][:]],
                        outs=[cc_out_h[hi][:]],
                    )

                def emit_oproj_partial(hi):
                    for t in range(ST):
                        ogc = cg.tile([128, NCORES, 512], bf16, tag="og", bufs=6, name=f"og{hi}{t}")
                        nc.sync.dma_start(
                            ogc[:], cc_out_h[hi][:, :, t * 512:(t + 1) * 512].transpose([1, 0, 2]))
                        for o in range(OCB):
                            o_ps = cpsum.tile([128, 512], f32, tag="ops", name=f"op{hi}{t}{o}")
                            for r in range(NCORES):
                                nc.tensor.matmul(o_ps[:], wo_s[:, hi * NCORES + r, o, :], ogc[:, r, :],
                                                 start=(r == 0), stop=(r == NCORES - 1))
                            if hi == 0:
                                nc.vector.tensor_copy(out_acc[:, t, o, :], o_ps[:])
                            else:
                                nc.vector.tensor_add(out_acc[:, t, o, :], out_acc[:, t, o, :], o_ps[:])
                            if hi == QH - 1:
                                nc.vector.tensor_scalar_add(out_acc[:, t, o, :], out_acc[:, t, o, :], bo_s[:, o:o + 1])
                                nc.sync.dma_start(out[o, t], out_acc[:, t, o, :])

                emit_attention(0)
                for cix in range(4):
                    nc.sync.dma_start(wo_s[:, cix * 8:(cix + 1) * 8, :, :],
                                      wot[:, cix * 8:(cix + 1) * 8, :, :])
                emit_attention(1)
                emit_attention(2)
                emit_attention(3)
                emit_oproj_partial(0)
                emit_oproj_partial(1)
                emit_oproj_partial(2)
                emit_oproj_partial(3)

    nc.finalize()
    return nc


def _pack_inputs(inputs):
    """Host-side shard + pack into DMA-friendly per-core layouts."""
    x = np.asarray(inputs["input_embeds"], np.float32).reshape(E, S)
    cos = np.asarray(inputs["cos"], np.float32)
    sin = np.asarray(inputs["sin"], np.float32)
    wq = np.asarray(inputs["wq"], np.float32)
    wk = np.asarray(inputs["wk"], np.float32)
    wv = np.asarray(inputs["wv"], np.float32)
    bv = np.asarray(inputs["bv"], np.float32)
    wo = np.asarray(inputs["wo"], np.float32)
    bo = np.asarray(inputs["bo"], np.float32)

    sinmod = np.concatenate([-sin[:64], sin[64:]], axis=0)

    # x packed: [ST, 128, EB, 512]; xp[t, p, b, s] = x[b*128+p, t*512+s]
    xp = np.ascontiguousarray(
        x.reshape(EB, 128, ST, 512).transpose(2, 1, 0, 3)
    ).astype(_BF16)

    # gathered-channel permutation for wo columns (hi-major after per-head AG):
    # g = hi*1024 + r*128 + d -> original channel (r + 8*hi)*128 + d
    g = np.arange(E)
    hi, rem = g // (NCORES * 128), g % (NCORES * 128)
    r, d = rem // 128, rem % 128
    colperm = (r + NCORES * hi) * 128 + d

    in_maps = []
    for c in range(NCORES):
        qheads = [c + NCORES * i for i in range(QH)]
        # wqt[p, b, hi, m] = wq[head*128+m, b*128+p]
        wq_loc = wq[np.concatenate([np.arange(h * 128, (h + 1) * 128) for h in qheads])]  # [512, E]
        wqt = np.ascontiguousarray(
            wq_loc.reshape(QH, 128, EB, 128).transpose(3, 2, 0, 1)
        ).astype(_BF16)
        wk_loc = wk[c * 128:(c + 1) * 128]  # [128, E]
        wkt = np.ascontiguousarray(
            wk_loc.reshape(128, EB, 128).transpose(2, 1, 0)
        ).astype(_BF16)
        wv_loc = wv[c * 128:(c + 1) * 128]
        wvt = np.ascontiguousarray(
            wv_loc.reshape(128, EB, 128).transpose(2, 1, 0)
        ).astype(_BF16)
        # wot[p, b, o, m] = wo[c*512 + o*128 + m, colperm[b*128+p]]
        wo_loc = wo[c * 512:(c + 1) * 512][:, colperm]  # [512, E] permuted cols
        wot = np.ascontiguousarray(
            wo_loc.reshape(OCB, 128, EB, 128).transpose(3, 2, 0, 1)
        ).astype(_BF16)
        in_maps.append({
            "xp": xp,
            "wqt": wqt, "wkt": wkt, "wvt": wvt, "wot": wot,
            "cosd": cos, "sind": sinmod,
            "bvp": np.ascontiguousarray(bv[c * 128:(c + 1) * 128].reshape(128, 1)),
            "bop": np.ascontiguousarray(bo[c * 512:(c + 1) * 512].reshape(OCB, 128).T),
        })
    return in_maps


def _install_ntff_hook():
    """The agent image lacks antenv.axon_hooks; recreate it so trace=True
    (neuron-profile exec_time_ns) works under axon."""
    import sys, types
    try:
        from antenv.axon_hooks import get_axon_ntff_profile_hook  # noqa
        return
    except ImportError:
        pass
    mod = types.ModuleType("antenv.axon_hooks")
    _h = [None]
    mod.set_axon_ntff_profile_hook = lambda h: _h.__setitem__(0, h)
    mod.get_axon_ntff_profile_hook = lambda: _h[0]
    sys.modules["antenv.axon_hooks"] = mod
    import antenv
    antenv.axon_hooks = mod
    try:
        from trn_agent_boot.trn_boot import _ntff_profile_via_ctypes
        mod.set_axon_ntff_profile_hook(
            _ntff_profile_via_ctypes("/opt/axon/libaxon_pjrt.so"))
    except Exception:
        pass


def kernel(_profile=False, **inputs):
    global _COMPILED, LAST_EXEC_NS
    from concourse.bass_utils import run_bass_kernel_spmd

    if _profile:
        _install_ntff_hook()

    if _COMPILED is None:
        _COMPILED = _build_graph()
    nc = _COMPILED

    in_maps = _pack_inputs(inputs)
    res = run_bass_kernel_spmd(nc, in_maps, core_ids=list(range(NCORES)), trace=_profile)
    if _profile:
        LAST_EXEC_NS = res.exec_time_ns
        global LAST_INSTS
        LAST_INSTS = res.instructions_and_trace
    outs = res.results

    full = np.empty((E, S), np.float32)
    for c in range(NCORES):
        oc = np.asarray(outs[c]["out"], np.float32)  # [OCB, ST, 128, 512]
        full[c * 512:(c + 1) * 512] = oc.transpose(0, 2, 1, 3).reshape(512, S)
    return full.reshape(B, E, 1, S)
